# revision 1
# baseline (speedup 1.0000x reference)
"""Correlation layer (FlowNet-style) Trainium2 Bass kernel.

Problem: in1, in2: [8, 256, 128, 128] fp32.
out[b, 9*dy+dx, y, x] = mean_c in1[b,c,y,x] * in2pad[b,c,y+dy,x+dx],
with in2 zero-padded by 4 on each spatial side, dy,dx in [0,9).
Output: [8, 81, 128, 128] fp32.

Sharding: data-parallel over batch -> 8 NeuronCores, one batch each
(SPMD: identical program, per-core input slices).

Per-core algorithm:
  Phase 1 (Gram matmuls), tiles of 128 output pixels (y-block 32 x x-block 4):
      stationary = in1[c, ytile, xtile]  (128 cols, x-outer/y-inner:
                                          i = x_off*32 + y_off)
      moving     = in2pad[c, y0:y0+40, x0:x0+12]  (480 cols, fp32r full rate)
      psum[i, j] = sum_c stat[c,i] * mov[c,j]  (accumulated over 2 c-blocks)
    The 81 correlation outputs of pixel i sit at j = (y_off+dy)*12+(x_off+dx),
    a sheared band.  Evacuate psum -> SBUF with *1/256; window-compact per
    32-partition group g (all pixels of a group share x_off=g, so the 9-col
    window [g, g+9) is partition-uniform; engine APs must start at partition
    0/32/64/96 which a 32-group satisfies).  In the compacted [40, 9] block
    the 81 useful values of pixel (g, u) are rows [u, u+9) = one contiguous
    324-byte run.  Dump to DRAM scratch.
  Phase 2: per-group DMA gather (flat DRAM addressing absorbs the
    partition-dependent run offset 9u), TensorE transpose
    [pixel, 81] -> [81, pixel], evacuate with the (x-outer,y-inner) -> (y,x)
    reorder, store [81, y, x] row-blocks.
"""

import numpy as np
from contextlib import ExitStack

import concourse.bacc as bacc
import concourse.tile as tile
import concourse.mybir as mybir
import concourse.bass as bass
from concourse import bass_utils

# ---- problem constants (hardcoded per contract) ----
B = 8
C = 256
H = W = 128
PAD = 4
D = 9            # displacements per axis
CH = D * D       # 81 output channels
HP = WP = H + 2 * PAD   # 136 padded

YB = 32          # y rows per tile
XBW = 4          # x cols per tile (stationary width)
MV_Y = YB + 8    # moving window rows   (40)
MV_X = XBW + 8   # moving window cols   (12)
N_YB = H // YB   # 4
N_XB = W // XBW  # 32
N_TILES = N_YB * N_XB   # 128
PSUM_F = MV_Y * MV_X    # 480 moving cols per tile
NG = 128 // YB   # 4 groups of 32 partitions per tile

# in2pad is held in SBUF as two y-halves (full padded tensor would not fit)
HALF_ROWS = 72   # padded rows per half: [0,72) and [64,136)

FP32 = mybir.dt.float32
FP32R = mybir.dt.float32r

USE_WINDOWS = True


def prep_in1(in1_b: np.ndarray) -> np.ndarray:
    """[C, H, W] -> [C, yb, x, y32]: makes each tile's stationary operand a
    contiguous 128-column slice (walrus requires single-free-dim weights)."""
    return np.ascontiguousarray(
        in1_b.reshape(C, N_YB, YB, W).swapaxes(2, 3)
    )


def build_nc():
    nc = bacc.Bacc("TRN2", target_bir_lowering=False, debug=False)
    in1_d = nc.dram_tensor("in1", [C, N_YB, W, YB], FP32, kind="ExternalInput").ap()
    in2_d = nc.dram_tensor("in2", [C, H, W], FP32, kind="ExternalInput").ap()
    out_d = nc.dram_tensor("out", [CH, H, W], FP32, kind="ExternalOutput").ap()
    if USE_WINDOWS:
        sdump_t = nc.dram_tensor("sdump", [N_TILES, 128, MV_Y, D], FP32, kind="Internal")
    else:
        sdump_t = nc.dram_tensor("sdump", [N_TILES, 128, MV_Y, MV_X], FP32, kind="Internal")
    sdump = sdump_t.ap()

    with tile.TileContext(nc) as tc, ExitStack() as es:
        const_pool = es.enter_context(tc.tile_pool(name="const", bufs=1))
        in2_pool = es.enter_context(tc.tile_pool(name="in2p", bufs=1))
        in1_pool = es.enter_context(tc.tile_pool(name="in1c", bufs=2))
        s_pool = es.enter_context(tc.tile_pool(name="sevac", bufs=3))
        w_pool = es.enter_context(tc.tile_pool(name="wcomp", bufs=4))
        t_pool = es.enter_context(tc.tile_pool(name="tgath", bufs=4))
        o_pool = es.enter_context(tc.tile_pool(name="oasm", bufs=3))
        psum_pool = es.enter_context(tc.tile_pool(name="psum", bufs=4, space="PSUM"))
        psum2_pool = es.enter_context(tc.tile_pool(name="psum2", bufs=2, space="PSUM"))

        # ---- identity matrix for TensorE transpose ----
        ones = const_pool.tile([128, 128], FP32, tag="ones")
        ident = const_pool.tile([128, 128], FP32, tag="ident")
        nc.gpsimd.memset(ones[:, :], 1.0)
        # iota[p, f] = f - p; ident = where(iota == 0, ones, 0)
        nc.gpsimd.affine_select(
            ident[:, :], ones[:, :], pattern=[[1, 128]],
            compare_op=mybir.AluOpType.is_equal, fill=0.0,
            base=0, channel_multiplier=-1,
        )

        # =========================== phase 1 ===========================
        for half in range(2):
            # padded rows [row0, row0+72) of in2pad live in SBUF this pass
            row0 = 0 if half == 0 else HP - HALF_ROWS  # 0 or 64
            in2p = in2_pool.tile([128, 2, HALF_ROWS, WP], FP32R, tag="in2p")
            # interior <- in2 rows [row0-4, row0+68-4) clipped to [0, 128)
            src_lo = max(row0 - PAD, 0)              # 0 / 60
            src_hi = min(row0 + HALF_ROWS - PAD, H)  # 68 / 128
            dst_lo = src_lo + PAD - row0             # 4 / 0
            dst_hi = dst_lo + (src_hi - src_lo)      # 72?no: 4+68=72 -> trimmed below
            # top/bottom zero rows within this half
            if dst_lo > 0:
                nc.vector.memset(in2p[:, :, 0:dst_lo, :].bitcast(FP32), 0.0)
            if dst_hi < HALF_ROWS:
                nc.vector.memset(in2p[:, :, dst_hi:HALF_ROWS, :].bitcast(FP32), 0.0)
            nc.gpsimd.memset(in2p[:, :, dst_lo:dst_hi, 0:PAD].bitcast(FP32), 0.0)
            nc.gpsimd.memset(in2p[:, :, dst_lo:dst_hi, WP - PAD:WP].bitcast(FP32), 0.0)
            for cb in range(2):
                nc.sync.dma_start(
                    in2p[:, cb, dst_lo:dst_hi, PAD:PAD + W],
                    in2_d[cb * 128:(cb + 1) * 128, src_lo:src_hi, :].bitcast(FP32R),
                )

            for yb in (0 + 2 * half, 1 + 2 * half):
                y0 = yb * YB             # global padded row of window start
                y0l = y0 - row0          # row within this half's SBUF tile
                in1c = in1_pool.tile([128, 2, W, YB], FP32R, tag="in1c")
                for cb in range(2):
                    nc.sync.dma_start(
                        in1c[:, cb, :, :],
                        in1_d[cb * 128:(cb + 1) * 128, yb, :, :].bitcast(FP32R),
                    )
                for xb in range(N_XB):
                    x0 = xb * XBW
                    t = yb * N_XB + xb
                    ps = psum_pool.tile([128, MV_Y, MV_X], FP32, tag="ps")
                    for cb in range(2):
                        stat = in1c[:, cb, x0:x0 + XBW, :].rearrange(
                            "p a b -> p (a b)"
                        )
                        mov = in2p[:, cb, y0l:y0l + MV_Y, x0:x0 + MV_X]
                        nc.tensor.matmul(
                            ps[:, :, :],
                            stat,
                            mov,
                            start=(cb == 0),
                            stop=(cb == 1),
                        )
                    # evacuate + scale (mean over C=256)
                    sv = s_pool.tile([128, MV_Y, MV_X], FP32, tag="sevac")
                    if t % 2 == 0:
                        nc.scalar.mul(sv[:, :, :], ps[:, :, :], 1.0 / C)
                    else:
                        nc.vector.tensor_scalar_mul(sv[:, :, :], ps[:, :, :], 1.0 / C)

                    if USE_WINDOWS:
                        wv = w_pool.tile([128, MV_Y, D], FP32, tag="wcomp")
                        for g in range(NG):
                            src = sv[32 * g:32 * (g + 1), :, g:g + D]
                            dst = wv[32 * g:32 * (g + 1), :, :]
                            e = (t + g) % 4
                            if e == 0:
                                nc.gpsimd.tensor_copy(dst, src)
                            elif e == 1:
                                nc.scalar.copy(dst, src)
                            else:
                                nc.vector.tensor_copy(dst, src)
                        nc.sync.dma_start(sdump[t], wv[:, :, :])
                    else:
                        nc.sync.dma_start(sdump[t], sv[:, :, :])

        # =========================== phase 2 ===========================
        for yb in range(N_YB):
            y0 = yb * YB
            oasm0 = o_pool.tile([128, YB // 2, W], FP32, tag="oasm")
            oasm1 = o_pool.tile([128, YB // 2, W], FP32, tag="oasm")
            oasm = [oasm0, oasm1]
            for xb in range(N_XB):
                x0 = xb * XBW
                t = yb * N_XB + xb
                tg = t_pool.tile([128, CH], FP32, tag="tgath")
                # gather the 81-value run of each pixel (flat DRAM addressing
                # absorbs the partition-dependent shear)
                for g in range(NG):
                    if USE_WINDOWS:
                        # elem offset for (u, k): (t*128 + 32g + u)*360 + 9u + k
                        base = (t * 128 + 32 * g) * (MV_Y * D)
                        src = bass.AP(sdump_t, base, [[MV_Y * D + D, 32], [1, CH]])
                        dst = tg[32 * g:32 * (g + 1), :]
                    else:
                        # elem offset (u, dy, dx):
                        #   (t*128 + 32g + u)*480 + (u+dy)*12 + (g+dx)
                        base = (t * 128 + 32 * g) * PSUM_F + g
                        src = bass.AP(
                            sdump_t, base,
                            [[PSUM_F + MV_X, 32], [MV_X, D], [1, D]],
                        )
                        dst = tg[32 * g:32 * (g + 1), :].rearrange(
                            "p (a b) -> p a b", a=D
                        )
                    nc.sync.dma_start(dst, src)
                # transpose [pixel, 81] -> [81, pixel]
                ps2 = psum2_pool.tile([128, XBW, YB], FP32, tag="ps2")
                nc.tensor.transpose(ps2[0:CH, :, :], tg[:, :], ident[:, :])
                # evacuate with (x-outer, y-inner) -> (y, x) reorder, y-halves
                for hf in range(2):
                    dst = oasm[hf][0:CH, :, x0:x0 + XBW].transpose([0, 2, 1])
                    src = ps2[0:CH, :, 16 * hf:16 * (hf + 1)]
                    if xb % 2 == 0:
                        nc.vector.tensor_copy(dst, src)
                    else:
                        nc.scalar.copy(dst, src)
            for hf in range(2):
                nc.sync.dma_start(
                    out_d[:, y0 + 16 * hf:y0 + 16 * (hf + 1), :],
                    oasm[hf][0:CH, :, :],
                )

    nc.compile()
    return nc


_NC_CACHE = None


def _get_nc():
    global _NC_CACHE
    if _NC_CACHE is None:
        _NC_CACHE = build_nc()
    return _NC_CACHE


def kernel(in1: np.ndarray, in2: np.ndarray) -> np.ndarray:
    nc = _get_nc()
    in1 = np.ascontiguousarray(np.asarray(in1, dtype=np.float32))
    in2 = np.ascontiguousarray(np.asarray(in2, dtype=np.float32))
    assert in1.shape == (B, C, H, W) and in2.shape == (B, C, H, W)
    in_maps = [{"in1": prep_in1(in1[b]), "in2": in2[b]} for b in range(B)]
    res = bass_utils.run_bass_kernel_spmd(nc, in_maps, core_ids=list(range(B)))
    out = np.stack([res.results[b]["out"] for b in range(B)], axis=0)
    return out



# revision 19
# speedup vs baseline: 2.7136x; 2.7136x over previous
"""Correlation layer (FlowNet-style) Trainium2 Bass kernel.

Problem: in1, in2: [8, 256, 128, 128] fp32.
out[b, 9*dy+dx, y, x] = mean_c in1[b,c,y,x] * in2pad[b,c,y+dy,x+dx],
with in2 zero-padded by 4 on each spatial side, dy,dx in [0,9).
Output: [8, 81, 128, 128] fp32.

Sharding: data-parallel over batch -> 8 NeuronCores, one batch each
(SPMD: identical program, per-core input slices).

Per-core algorithm (all-bf16 datapath; 1/256 mean folded into in1 on host):
  Phase 1 (Gram matmuls), tiles of 128 output pixels (y-block 32 x x-block 4):
      stationary = in1[c, ytile, xtile]  (128 cols, x-outer/y-inner:
                                          i = x_off*32 + y_off)
      moving     = in2pad[c, y0:y0+40, x0:x0+12]  (480 cols)
      psum[i, j] = sum_c stat[c,i] * mov[c,j]  (accumulated over 2 c-blocks)
    The 81 correlation outputs of pixel i sit at j = (y_off+dy)*12+(x_off+dx),
    a sheared band.  Window-compact straight out of PSUM (cast fp32->bf16)
    per 32-partition group g (all pixels of a group share x_off=g, so the
    9-col window [g, g+9) is partition-uniform).  In the compacted [40, 9]
    block the 81 useful values of pixel (g, u) are one contiguous run at
    offset 9u.  Batches of KB=8 tiles dump to a per-batch DRAM scratch
    (one DMA per batch).
  Phase 2 per batch: one 4D-AP DMA gather (flat DRAM addressing absorbs the
    partition-dependent run offset 9u), TensorE transpose
    [pixel, 81] -> [81, pixel] per tile, evacuate with the
    (x-outer,y-inner) -> (y,x) reorder into a per-yb row-block, store
    [81, 32, 128] bf16 row-blocks (host upcasts to fp32).
"""

import numpy as np
from contextlib import ExitStack

import ml_dtypes

import concourse.bacc as bacc
import concourse.tile as tile
import concourse.mybir as mybir
import concourse.bass as bass
from concourse import bass_utils

# ---- problem constants (hardcoded per contract) ----
B = 8
C = 256
H = W = 128
PAD = 4
D = 9            # displacements per axis
CH = D * D       # 81 output channels
HP = WP = H + 2 * PAD   # 136 padded

YB = 32          # y rows per tile
XBW = 4          # x cols per tile (stationary width)
MV_Y = YB + 8    # moving window rows   (40)
MV_X = XBW + 8   # moving window cols   (12)
N_YB = H // YB   # 4
N_XB = W // XBW  # 32
N_TILES = N_YB * N_XB   # 128
NG = 128 // YB   # 4 groups of 32 partitions per tile
WIN = MV_Y * D   # 360 compacted window elems per pixel

KB = 16                  # tiles per scratch batch (last y-block tapers)
# per-y-block batch sizes: taper the final y-block so the drain tail
# (last dump -> gather -> transpose -> out) is short
YB_BATCHES = [[16, 16], [16, 16], [16, 16], [16, 8, 4, 4]]
NBATCH = sum(len(b) for b in YB_BATCHES)

FP32 = mybir.dt.float32
BF16 = mybir.dt.bfloat16
NPBF16 = ml_dtypes.bfloat16


def prep_inputs(in1: np.ndarray, in2: np.ndarray) -> list[dict]:
    """Host-side prep: tile layout + 1/256 prescale for in1, zero-pad for
    in2, both cast to bf16.  Returns per-core input maps."""
    # [B, C, H, W] -> [B, C, yb, x, y32] -> flat [B, C, yb, 4096], col x*32+y
    in1p = (in1.reshape(B, C, N_YB, YB, W).swapaxes(3, 4)
            * np.float32(1.0 / C)).astype(NPBF16)
    in1p = np.ascontiguousarray(in1p.reshape(B, C, N_YB, W * YB))
    in2p = np.zeros((B, C, HP, WP), dtype=NPBF16)
    in2p[:, :, PAD:PAD + H, PAD:PAD + W] = in2.astype(NPBF16)
    return [{"in1": in1p[b], "in2": in2p[b]} for b in range(B)]


def build_nc():
    nc = bacc.Bacc("TRN2", target_bir_lowering=False, debug=False)
    in1_t = nc.dram_tensor("in1", [C, N_YB, W * YB], BF16, kind="ExternalInput")
    in2_t = nc.dram_tensor("in2", [C, HP, WP], BF16, kind="ExternalInput")
    out_d = nc.dram_tensor("out", [CH, H, W], BF16, kind="ExternalOutput").ap()
    # scratch row pitch 369 (= WIN + D) and per-pixel block pitch
    # 16*369 - 9 = 5895: row (p, kb) lives at p*5895 + kb*369.  The gather
    # for pixel p reads [9u, 9u+81) of each row, so its (u, kb) dims have
    # strides 5904 = 16*369 and 369 -> they merge into one 512-count dim,
    # keeping the gather AP at 3 dims.  The 9-elem row slack overlaps the
    # NEXT pixel's first row but is never written (rows are 360 long).
    RPITCH = WIN + D            # 369
    KBS = [kb for b in YB_BATCHES for kb in b]

    def ppitch(kbn):
        return kbn * RPITCH - D

    sd_t = [
        nc.dram_tensor(
            f"sd{j}",
            [127 * ppitch(kbn) + (kbn - 1) * RPITCH + WIN],
            BF16, kind="Internal",
        )
        for j, kbn in enumerate(KBS)
    ]

    # element strides of the dram input layouts
    S1_C, S1_YB = N_YB * W * YB, W * YB          # in1 [C, 4, 4096]
    S2_C = HP * WP                               # in2 [C, 136, 136]

    with tile.TileContext(nc) as tc, ExitStack() as es:
        const_pool = es.enter_context(tc.tile_pool(name="const", bufs=1))
        in1_pool = es.enter_context(tc.tile_pool(name="in1p", bufs=3))
        in2_pool = es.enter_context(tc.tile_pool(name="in2p", bufs=3))
        wv_pool = es.enter_context(tc.tile_pool(name="wv", bufs=2))
        sv_pool = es.enter_context(tc.tile_pool(name="sv", bufs=3))
        tg_pool = es.enter_context(tc.tile_pool(name="tg", bufs=2))
        o_pool = es.enter_context(tc.tile_pool(name="oasm", bufs=2))
        ps_pool = es.enter_context(tc.tile_pool(name="ps", bufs=4, space="PSUM"))
        ps2_pool = es.enter_context(tc.tile_pool(name="ps2", bufs=2, space="PSUM"))

        # ---- identity matrix (bf16) for TensorE transpose ----
        ones = const_pool.tile([128, 128], FP32, tag="ones")
        identf = const_pool.tile([128, 128], FP32, tag="identf")
        ident = const_pool.tile([128, 128], BF16, tag="ident")
        nc.gpsimd.memset(ones[:, :], 1.0)
        # iota[p, f] = f - p; ident = where(iota == 0, ones, 0)
        nc.gpsimd.affine_select(
            identf[:, :], ones[:, :], pattern=[[1, 128]],
            compare_op=mybir.AluOpType.is_equal, fill=0.0,
            base=0, channel_multiplier=-1,
        )
        nc.vector.tensor_copy(ident[:, :], identf[:, :])

        # ---- per-y-block input tiles (bufs=2 pools stagger the loads:
        # yb+2's load waits on yb's last consumer via buffer reuse, so dumps
        # and gathers interleave with loads on the DMA engines) ----
        ybtiles = {}

        def issue_loads(yb):
            in1t = in1_pool.tile([128, 2, W * YB], BF16, tag="in1t")
            in2t = in2_pool.tile([128, 2, MV_Y, WP], BF16, tag="in2t")
            # rows [0, 8) of this window = rows [32, 40) of the previous one:
            # copy them SBUF->SBUF on idle gpsimd instead of re-reading HBM
            r0 = 0 if yb == 0 else 8
            if yb > 0:
                prev = ybtiles[yb - 1][1]
                nc.gpsimd.tensor_copy(in2t[:, :, 0:8, :], prev[:, :, YB:MV_Y, :])
            for cb in range(2):
                nc.sync.dma_start(
                    in1t[:, cb, :],
                    bass.AP(in1_t, cb * 128 * S1_C + yb * S1_YB,
                            [[S1_C, 128], [1, W * YB]]),
                )
                nc.sync.dma_start(
                    in2t[:, cb, r0:MV_Y, :],
                    bass.AP(in2_t, cb * 128 * S2_C + (yb * YB + r0) * WP,
                            [[S2_C, 128], [WP, MV_Y - r0], [1, WP]]),
                )
            ybtiles[yb] = (in1t, in2t)

        issue_loads(0)

        for yb in range(N_YB):
            if yb + 1 < N_YB:
                issue_loads(yb + 1)
            in1t, in2t = ybtiles[yb]
            oasm = o_pool.tile([128, YB, W], BF16, tag="oasm")
            xb_base = 0
            for bj, KBN in enumerate(YB_BATCHES[yb]):
                j = sum(len(b) for b in YB_BATCHES[:yb]) + bj
                PPITCH = ppitch(KBN)
                wv = wv_pool.tile([128, KBN, MV_Y, D], BF16, tag="wv")
                # ---------------- phase 1: KBN tiles ----------------
                for kb in range(KBN):
                    xb = xb_base + kb
                    x0 = xb * XBW
                    t = yb * N_XB + xb
                    ps = ps_pool.tile([128, MV_Y, MV_X], FP32, tag="ps")
                    for cb in range(2):
                        nc.tensor.matmul(
                            ps[:, :, :],
                            in1t[:, cb, xb * 128:(xb + 1) * 128],
                            in2t[:, cb, :, x0:x0 + MV_X],
                            start=(cb == 0),
                            stop=(cb == 1),
                        )
                    # evacuate PSUM once (cast to bf16), then window-compact
                    # SBUF->SBUF on DVE where all-bf16 packed operands hit the
                    # 4x perf mode
                    sv = sv_pool.tile([128, MV_Y, MV_X], BF16, tag="sv")
                    nc.scalar.copy(sv[:, :, :], ps[:, :, :])
                    for g in range(NG):
                        src = sv[32 * g:32 * (g + 1), :, g:g + D]
                        dst = wv[32 * g:32 * (g + 1), kb, :, :]
                        nc.vector.tensor_copy(dst, src)
                # batch dump: row (p, kb) at p*PPITCH + kb*369, 360 elems
                nc.sync.dma_start(
                    bass.AP(sd_t[j], 0,
                            [[PPITCH, 128], [RPITCH, KBN], [1, WIN]]),
                    wv[:, :, :, :],
                )
                # ---------------- phase 2: gather + transpose ----------------
                tg = tg_pool.tile([128, KBN, CH], BF16, tag="tg")
                # elem offset for (g, u, kb, k):
                #   g*32*PPITCH + u*(PPITCH+9) + kb*369 + k ((u, kb) merged)
                nc.sync.dma_start(
                    tg[:, :, :],
                    bass.AP(sd_t[j], 0,
                            [[32 * PPITCH, NG], [RPITCH, 32 * KBN], [1, CH]]),
                )
                # 4 transposes share one PSUM bank, then one merged evacuate
                for kq in range(KBN // 4):
                    ps2 = ps2_pool.tile([128, 4, XBW, YB], BF16, tag="ps2")
                    for kk in range(4):
                        kb = 4 * kq + kk
                        nc.tensor.transpose(
                            ps2[0:CH, kk, :, :], tg[:, kb, :], ident[:, :]
                        )
                    # evacuate with (kq,x-outer,y-inner) -> (y, x) reorder
                    x0 = (xb_base + 4 * kq) * XBW
                    dst = oasm[0:CH, :, x0:x0 + 4 * XBW].rearrange(
                        "p y (kk x) -> p y kk x", kk=4
                    ).transpose([0, 2, 3, 1])
                    src = ps2[0:CH, :, :, :]
                    if kq % 2 == 0:
                        nc.vector.tensor_copy(dst, src)
                    else:
                        nc.scalar.copy(dst, src)
                xb_base += KBN
            nc.gpsimd.dma_start(out_d[:, yb * YB:(yb + 1) * YB, :], oasm[0:CH, :, :])

    nc.compile()
    return nc


_NC_CACHE = None


def _get_nc():
    global _NC_CACHE
    if _NC_CACHE is None:
        _NC_CACHE = build_nc()
    return _NC_CACHE


def kernel(in1: np.ndarray, in2: np.ndarray) -> np.ndarray:
    nc = _get_nc()
    in1 = np.asarray(in1, dtype=np.float32)
    in2 = np.asarray(in2, dtype=np.float32)
    assert in1.shape == (B, C, H, W) and in2.shape == (B, C, H, W)
    in_maps = prep_inputs(in1, in2)
    res = bass_utils.run_bass_kernel_spmd(nc, in_maps, core_ids=list(range(B)))
    out = np.stack([res.results[b]["out"] for b in range(B)], axis=0)
    return out.astype(np.float32)
